# revision 14
# baseline (speedup 1.0000x reference)
"""Trainium2 Bass kernel for nn_Attention_65128884077225.

Math: the reference module broadcasts scores [B,H,S,1] along the softmax
axis, so every softmax row is constant -> attention weights are exactly
uniform (1/S). Hence z = mean_s(v) broadcast over s, and the whole module
collapses to, per batch b:

    c[b] = (mean_s x[b,s,:]) @ Wv @ Wout + (bv @ Wout + bout)
    out[b,s,:] = c[b]                      (constant across s)

where Wv = qkv_w[:, 2E:3E], bv = qkv_b[2E:3E].

Sharding: 8 cores = 4 batches x 2 column-halves. Core c handles batch
b=c//2 and output columns [h*256, (h+1)*256), h=c%2.

Device kernel per core — HYBRID reduction so neither engine is the tail:
  - rows 0:1024 of x[b] arrive TRANSPOSED [4, 128, 1024] (E-major); DVE
    free-dim tensor_reduce gives xsum^T columns directly,
  - rows 1024:2048 arrive NATURAL [1024, 512]; the PE row-reduces them
    with accumulating ones-vector matmuls -> [1,512] PSUM row, then 4
    tiny matmuls transpose that row into [128,4] columns (all during the
    read stream),
  - weights Wc (bf16) + bias arrive on the SWDGE queue (gpsimd),
  - crow = bias + sum_cE (xsumT_dve+xsumT_pe)[cE] @ Wc[cE] (bf16 PSUM
    chain, bias matmul first = off the tail),
  - two rank-1 broadcast matmuls into TWO PSUM banks; DVE and ACT each
    replicate 2 copies in parallel -> [128, 4, 256],
  - ONE store: dst p-major [128, 16*256] (16 KiB/partition contiguous),
    src stride-0 over 4 groups -> 4 KiB descriptors.

Host only: fold Wc = (Wv @ Wout)/S and bc = bv @ Wout + bout (tiny host
GEMM), transpose/shard inputs, un-transpose the per-core outputs.
"""

import sys

import numpy as np

if "/opt/trn_rl_repo" not in sys.path and not any(
    p.endswith("trn_rl_repo") for p in sys.path
):
    sys.path.insert(0, "/opt/trn_rl_repo")

import ml_dtypes

import concourse.bacc as bacc
import concourse.mybir as mybir
import concourse.tile as tile
from concourse.bass_utils import run_bass_kernel_spmd

B, S, E = 4, 2048, 512
N_CORES = 8
P = 128
EH = E // 2            # 256 output columns per core
NEC = E // P           # 4 E-chunks of 128 rows of x^T
SD = 1536              # rows to DVE (transposed); rest natural to the PE
SN = S - SD            # 512 rows to PE
SBIG = 1024            # big/small split of each transposed E-chunk
FP32 = mybir.dt.float32
BF16 = mybir.dt.bfloat16
BCAST_Q = 4            # SBUF-side replication of the out tile

_CACHE = {}


def build():
    """Build + compile the per-core Bass program (same for every core)."""
    if "nc" in _CACHE:
        return _CACHE["nc"]
    nc = bacc.Bacc(None, target_bir_lowering=False, enable_partition_id=False)
    xt_d = nc.dram_tensor("xt", [NEC, P, SD], FP32, kind="ExternalInput")
    xn_d = nc.dram_tensor("xn", [SN, E], FP32, kind="ExternalInput")
    w_d = nc.dram_tensor("w", [P, NEC * EH], BF16, kind="ExternalInput")
    b_d = nc.dram_tensor("b", [1, EH], BF16, kind="ExternalInput")
    o_d = nc.dram_tensor("o", [P, (S // P) * EH], FP32, kind="ExternalOutput")
    xn_v = xn_d.rearrange("(g p) e -> p g e", p=P)

    with tile.TileContext(nc) as tc:
        with (
            tc.tile_pool(name="xp", bufs=12) as xp,
            tc.tile_pool(name="wp", bufs=1) as wp,
            tc.tile_pool(name="sp", bufs=1) as sp,
            tc.tile_pool(name="ps", bufs=1, space="PSUM") as ps,
        ):
            one1 = sp.tile([1, 1], BF16, tag="one1")
            nc.vector.memset(one1[:], 1.0)
            onesc = sp.tile([P, 1], FP32, tag="onesc")
            nc.vector.memset(onesc[:], 1.0)
            ones_row = sp.tile([1, P], BF16, tag="ones_row")
            nc.vector.memset(ones_row[:], 1.0)

            # weights + bias FIRST on the HWDGE rings (small; the bias
            # matmul heads the in-order PE stream so it must land early)
            wcb = wp.tile([P, NEC * EH], BF16, tag="w")
            nc.sync.dma_start(wcb[:], w_d[:, :])
            brow = sp.tile([1, EH], BF16, tag="brow")
            nc.scalar.dma_start(brow[:], b_d[:, :])

            p_crow = ps.tile([1, EH], FP32, tag="crow")
            nc.tensor.matmul(p_crow[:], one1[:], brow[:], start=True, stop=False)

            # ring schedules; trailing transposed chunks are small so the
            # last reduces are short
            nts, tts = {}, {}
            plan = [
                (nc.sync, "t", (0, 0, SBIG)), (nc.scalar, "t", (1, 0, SBIG)),
                (nc.sync, "n", 0), (nc.scalar, "n", 1),
                (nc.sync, "t", (2, 0, SBIG)), (nc.scalar, "t", (3, 0, SBIG)),
                (nc.sync, "t", (0, SBIG, SD)), (nc.scalar, "t", (1, SBIG, SD)),
                (nc.sync, "t", (2, SBIG, SD)), (nc.scalar, "t", (3, SBIG, SD)),
            ]
            for eng, kind, arg in plan:
                if kind == "n":
                    g = arg
                    nt = xp.tile([P, 2, E], FP32, tag="xn")
                    eng.dma_start(nt[:], xn_v[:, 2 * g : 2 * g + 2, :])
                    nts[g] = nt
                else:
                    cE, s0, s1 = arg
                    tt = xp.tile([P, s1 - s0], FP32, tag="xt")
                    eng.dma_start(tt[:], xt_d[cE, :, s0:s1])
                    tts[arg] = tt

            # PE row-reduction of the natural tiles (in arrival order)
            p_row = ps.tile([1, E], FP32, tag="prow")
            for i, g in enumerate([0, 1]):
                for t in range(2):
                    nc.tensor.matmul(
                        p_row[:],
                        onesc[:],
                        nts[g][:, t, :],
                        start=(i == 0 and t == 0),
                        stop=(i == 1 and t == 1),
                    )
            row_sb = sp.tile([1, E], BF16, tag="row_sb")
            nc.vector.tensor_copy(row_sb[:], p_row[:])
            p_xt = ps.tile([P, NEC], FP32, tag="pxt")
            for cE in range(NEC):
                nc.tensor.matmul(
                    p_xt[:, cE : cE + 1],
                    row_sb[0:1, cE * P : (cE + 1) * P],
                    one1[:],
                    start=True,
                    stop=True,
                )

            # DVE reduction of the transposed chunks (in arrival order),
            # then per-E-chunk combine + cast + crow matmul as soon as that
            # chunk's small tail arrives
            part = sp.tile([P, NEC, 2], FP32, tag="part")
            xsT_f = sp.tile([P, NEC], FP32, tag="xsT_f")
            xsT_b = sp.tile([P, NEC], BF16, tag="xsT_b")
            for cE in range(NEC):
                nc.vector.tensor_reduce(
                    part[:, cE, 0:1],
                    tts[(cE, 0, SBIG)][:],
                    axis=mybir.AxisListType.X,
                    op=mybir.AluOpType.add,
                )
            for cE in range(NEC):
                nc.vector.tensor_reduce(
                    part[:, cE, 1:2],
                    tts[(cE, SBIG, SD)][:],
                    axis=mybir.AxisListType.X,
                    op=mybir.AluOpType.add,
                )
                nc.vector.tensor_add(
                    xsT_f[:, cE : cE + 1], part[:, cE, 0:1], part[:, cE, 1:2]
                )
                nc.vector.tensor_add(
                    xsT_f[:, cE : cE + 1],
                    xsT_f[:, cE : cE + 1],
                    p_xt[:, cE : cE + 1],
                )
                nc.vector.tensor_copy(xsT_b[:, cE : cE + 1], xsT_f[:, cE : cE + 1])
                nc.tensor.matmul(
                    p_crow[:],
                    xsT_b[:, cE : cE + 1],
                    wcb[:, cE * EH : (cE + 1) * EH],
                    start=False,
                    stop=(cE == NEC - 1),
                )

            crow_b = sp.tile([1, EH], BF16, tag="crow_b")
            nc.vector.tensor_copy(crow_b[:], p_crow[:])

            # two broadcast matmuls into two PSUM banks -> DVE and ACT
            # replicate in parallel without PSUM port serialization
            p_bc0 = ps.tile([P, EH], FP32, tag="bc0")
            p_bc1 = ps.tile([P, EH], FP32, tag="bc1")
            nc.tensor.matmul(p_bc0[:], ones_row[:], crow_b[:], start=True, stop=True)
            nc.tensor.matmul(p_bc1[:], ones_row[:], crow_b[:], start=True, stop=True)
            bcast = sp.tile([P, BCAST_Q, EH], FP32, tag="bcast")
            nc.vector.tensor_copy(
                bcast[:, 0:2, :], p_bc0[:, None, :].broadcast_to([P, 2, EH])
            )
            nc.scalar.copy(bcast[:, 2, :], p_bc1[:, :])
            nc.scalar.copy(bcast[:, 3, :], p_bc1[:, :])

            o_t = o_d.rearrange("p (g q e) -> p g (q e)", q=BCAST_Q, e=EH)
            src = bcast[:, None, :, :].broadcast_to(
                [P, (S // P) // BCAST_Q, BCAST_Q, EH]
            ).rearrange("p g q e -> p g (q e)")
            nc.sync.dma_start(o_t[:, :, :], src)

    nc.compile()
    _CACHE["nc"] = nc
    return nc


def _fold_weights(qkv_w, qkv_b, out_w, out_b):
    wv = np.asarray(qkv_w)[:, 2 * E : 3 * E].astype(np.float64)
    wc = (wv @ np.asarray(out_w).astype(np.float64) / S).astype(np.float32)
    bc = (
        np.asarray(qkv_b)[2 * E : 3 * E].astype(np.float64)
        @ np.asarray(out_w).astype(np.float64)
        + np.asarray(out_b)
    ).astype(np.float32)
    return wc, bc


def _pack_w(wc, h):
    """[128, 4*256] bf16: E-chunk-major packing of this half's Wc columns."""
    cols = slice(h * EH, (h + 1) * EH)
    return np.ascontiguousarray(
        wc[:, cols].reshape(NEC, P, EH).transpose(1, 0, 2).reshape(P, NEC * EH)
    ).astype(ml_dtypes.bfloat16)


def _run(inputs, trace=False, **kwargs):
    nc = build()
    x = np.asarray(inputs["x"], dtype=np.float32)
    xT = [np.ascontiguousarray(x[b, :SD].T.reshape(NEC, P, SD)) for b in range(B)]
    xN = [np.ascontiguousarray(x[b, SD:]) for b in range(B)]
    assert xN[0].shape == (SN, E)
    wc, bc = _fold_weights(
        inputs["qkv_w"], inputs["qkv_b"], inputs["out_w"], inputs["out_b"]
    )
    wpk = [_pack_w(wc, h) for h in range(2)]
    bpk = [
        np.ascontiguousarray(bc[h * EH : (h + 1) * EH].reshape(1, EH)).astype(
            ml_dtypes.bfloat16
        )
        for h in range(2)
    ]
    in_maps = [
        {"xt": xT[c // 2], "xn": xN[c // 2], "w": wpk[c % 2], "b": bpk[c % 2]}
        for c in range(N_CORES)
    ]
    res = run_bass_kernel_spmd(
        nc, in_maps, core_ids=list(range(N_CORES)), trace=trace, **kwargs
    )
    out = np.empty((B, S, E), dtype=np.float32)
    for b in range(B):
        for h in range(2):
            o = res.results[2 * b + h]["o"]
            o = o.reshape(P, S // P, EH).transpose(1, 0, 2).reshape(S, EH)
            out[b, :, h * EH : (h + 1) * EH] = o
    return out, res


def kernel(**inputs) -> np.ndarray:
    out, _ = _run(inputs, trace=False)
    return out


# revision 15
# speedup vs baseline: 1.0112x; 1.0112x over previous
"""Trainium2 Bass kernel for nn_Attention_65128884077225.

Math: the reference module broadcasts scores [B,H,S,1] along the softmax
axis, so every softmax row is constant -> attention weights are exactly
uniform (1/S). Hence z = mean_s(v) broadcast over s, and the whole module
collapses to, per batch b:

    c[b] = (mean_s x[b,s,:]) @ Wv @ Wout + (bv @ Wout + bout)
    out[b,s,:] = c[b]                      (constant across s)

where Wv = qkv_w[:, 2E:3E], bv = qkv_b[2E:3E].

Sharding: 8 cores = 4 batches x 2 column-halves. Core c handles batch
b=c//2 and output columns [h*256, (h+1)*256), h=c%2.

Device kernel per core (all layouts pre-arranged on host):
  - weights Wc (bf16) + bias lead the two HWDGE rings (the bias matmul
    heads the in-order PE stream, so it must land early),
  - x[b] arrives TRANSPOSED [4, 128, 2048] (E-major) as 12 chunks
    (4x512K + 8x256K, small ones last so the final reduce is short),
  - each chunk is row-summed by one DVE free-dim tensor_reduce; xsum^T
    lands directly as [128,1] columns; per-E-chunk combine + cast + crow
    matmul fire as soon as that chunk's last piece lands,
  - crow = bias + sum_cE xsumT[cE] @ Wc[cE] (bf16 matmuls, fp32 PSUM);
    dummy warm matmuls gated on late chunks keep the PE clock high,
  - two rank-1 broadcast matmuls into TWO PSUM banks; DVE and ACT
    replicate 2 copies each in parallel -> [128, 4, 256],
  - ONE store: dst p-major [128, 16*256] (16 KiB/partition contiguous),
    src stride-0 over 4 groups -> 4 KiB descriptors.

Host only: fold Wc = (Wv @ Wout)/S and bc = bv @ Wout + bout (tiny host
GEMM), transpose/shard inputs, un-transpose the per-core outputs.
"""

import sys

import numpy as np

if "/opt/trn_rl_repo" not in sys.path and not any(
    p.endswith("trn_rl_repo") for p in sys.path
):
    sys.path.insert(0, "/opt/trn_rl_repo")

import ml_dtypes

import concourse.bacc as bacc
import concourse.mybir as mybir
import concourse.tile as tile
from concourse.bass_utils import run_bass_kernel_spmd

B, S, E = 4, 2048, 512
N_CORES = 8
P = 128
EH = E // 2            # 256 output columns per core
NEC = E // P           # 4 E-chunks of 128 rows of x^T
SA = 1024              # big chunk (a); then two 512 smalls (b, c)
SB = 1536
FP32 = mybir.dt.float32
BF16 = mybir.dt.bfloat16
BCAST_Q = 4            # SBUF-side replication of the out tile

_CACHE = {}


def build():
    """Build + compile the per-core Bass program (same for every core)."""
    if "nc" in _CACHE:
        return _CACHE["nc"]
    nc = bacc.Bacc(None, target_bir_lowering=False, enable_partition_id=False)
    xt_d = nc.dram_tensor("xt", [NEC, P, S], FP32, kind="ExternalInput")
    w_d = nc.dram_tensor("w", [P, NEC * EH], BF16, kind="ExternalInput")
    b_d = nc.dram_tensor("b", [1, EH], BF16, kind="ExternalInput")
    o_d = nc.dram_tensor("o", [P, (S // P) * EH], FP32, kind="ExternalOutput")

    with tile.TileContext(nc) as tc:
        with (
            tc.tile_pool(name="xp", bufs=12) as xp,
            tc.tile_pool(name="wp", bufs=1) as wp,
            tc.tile_pool(name="sp", bufs=1) as sp,
            tc.tile_pool(name="ps", bufs=1, space="PSUM") as ps,
        ):
            one1 = sp.tile([1, 1], BF16, tag="one1")
            nc.vector.memset(one1[:], 1.0)
            onesc = sp.tile([P, 1], FP32, tag="onesc")
            nc.vector.memset(onesc[:], 1.0)
            ones_row = sp.tile([1, P], BF16, tag="ones_row")
            nc.vector.memset(ones_row[:], 1.0)

            # weights + bias first on the HWDGE rings (small)
            wcb = wp.tile([P, NEC * EH], BF16, tag="w")
            nc.sync.dma_start(wcb[:], w_d[:, :])
            brow = sp.tile([1, EH], BF16, tag="brow")
            nc.scalar.dma_start(brow[:], b_d[:, :])

            p_crow = ps.tile([1, EH], FP32, tag="crow")
            nc.tensor.matmul(p_crow[:], one1[:], brow[:], start=True, stop=False)
            p_warm = ps.tile([1, 64], FP32, tag="warm")

            # 12 x chunks: (cE, piece) with piece a=[0:1024], b=[1024:1536],
            # c=[1536:2048]; ring plan keeps both rings at 2.0-2.25 MB and
            # delivers pieces in emission order
            pieces = {"a": (0, SA), "b": (SA, SB), "c": (SB, S)}
            plan = [
                (nc.sync, 0, "a"), (nc.scalar, 1, "a"),
                (nc.sync, 2, "a"), (nc.scalar, 3, "a"),
                (nc.scalar, 0, "b"), (nc.sync, 1, "b"),
                (nc.scalar, 2, "b"), (nc.sync, 3, "b"),
                (nc.scalar, 0, "c"), (nc.sync, 1, "c"),
                (nc.scalar, 2, "c"), (nc.sync, 3, "c"),
            ]
            tts = {}
            for eng, cE, pc in plan:
                s0, s1 = pieces[pc]
                tt = xp.tile([P, s1 - s0], FP32, tag="xt")
                eng.dma_start(tt[:], xt_d[cE, :, s0:s1])
                tts[(cE, pc)] = tt

            part = sp.tile([P, NEC, 2], FP32, tag="part")
            xsT_f = sp.tile([P, NEC], FP32, tag="xsT_f")
            xsT_b = sp.tile([P, NEC], BF16, tag="xsT_b")

            # reduces in arrival order; combine + crow matmul per E-chunk
            # as its last piece lands; warm matmuls keep the PE clocked
            arrival = [(cE, "a") for cE in range(NEC)] + [
                (cE, pc) for pc in ("b", "c") for cE in (0, 1, 2, 3)
            ]
            last_cE = 3
            for cE, pc in arrival:
                tt = tts[(cE, pc)]
                if pc == "a":
                    nc.vector.tensor_reduce(
                        part[:, cE, 0:1], tt[:],
                        axis=mybir.AxisListType.X, op=mybir.AluOpType.add,
                    )
                    continue
                if pc == "b":
                    nc.vector.tensor_reduce(
                        part[:, cE, 1:2], tt[:],
                        axis=mybir.AxisListType.X, op=mybir.AluOpType.add,
                    )
                    nc.vector.tensor_add(
                        part[:, cE, 0:1], part[:, cE, 0:1], part[:, cE, 1:2]
                    )
                    # PE keep-warm, gated on this chunk's arrival
                    nc.tensor.matmul(
                        p_warm[:], onesc[:], tt[:, 0:64], start=True, stop=True
                    )
                    continue
                nc.vector.tensor_reduce(
                    part[:, cE, 1:2], tt[:],
                    axis=mybir.AxisListType.X, op=mybir.AluOpType.add,
                )
                nc.vector.tensor_add(
                    xsT_f[:, cE : cE + 1], part[:, cE, 0:1], part[:, cE, 1:2]
                )
                nc.vector.tensor_copy(xsT_b[:, cE : cE + 1], xsT_f[:, cE : cE + 1])
                nc.tensor.matmul(
                    p_crow[:],
                    xsT_b[:, cE : cE + 1],
                    wcb[:, cE * EH : (cE + 1) * EH],
                    start=False,
                    stop=(cE == last_cE),
                )

            crow_b = sp.tile([1, EH], BF16, tag="crow_b")
            nc.vector.tensor_copy(crow_b[:], p_crow[:])

            # two broadcast matmuls into two PSUM banks -> DVE and ACT
            # replicate in parallel without PSUM port serialization
            p_bc0 = ps.tile([P, EH], FP32, tag="bc0")
            p_bc1 = ps.tile([P, EH], FP32, tag="bc1")
            nc.tensor.matmul(p_bc0[:], ones_row[:], crow_b[:], start=True, stop=True)
            nc.tensor.matmul(p_bc1[:], ones_row[:], crow_b[:], start=True, stop=True)
            bcast = sp.tile([P, BCAST_Q, EH], FP32, tag="bcast")
            nc.vector.tensor_copy(
                bcast[:, 0:2, :], p_bc0[:, None, :].broadcast_to([P, 2, EH])
            )
            nc.scalar.copy(bcast[:, 2, :], p_bc1[:, :])
            nc.scalar.copy(bcast[:, 3, :], p_bc1[:, :])

            o_t = o_d.rearrange("p (g q e) -> p g (q e)", q=BCAST_Q, e=EH)
            src = bcast[:, None, :, :].broadcast_to(
                [P, (S // P) // BCAST_Q, BCAST_Q, EH]
            ).rearrange("p g q e -> p g (q e)")
            nc.sync.dma_start(o_t[:, :, :], src)

    nc.compile()
    _CACHE["nc"] = nc
    return nc


def _fold_weights(qkv_w, qkv_b, out_w, out_b):
    wv = np.asarray(qkv_w)[:, 2 * E : 3 * E].astype(np.float64)
    wc = (wv @ np.asarray(out_w).astype(np.float64) / S).astype(np.float32)
    bc = (
        np.asarray(qkv_b)[2 * E : 3 * E].astype(np.float64)
        @ np.asarray(out_w).astype(np.float64)
        + np.asarray(out_b)
    ).astype(np.float32)
    return wc, bc


def _pack_w(wc, h):
    """[128, 4*256] bf16: E-chunk-major packing of this half's Wc columns."""
    cols = slice(h * EH, (h + 1) * EH)
    return np.ascontiguousarray(
        wc[:, cols].reshape(NEC, P, EH).transpose(1, 0, 2).reshape(P, NEC * EH)
    ).astype(ml_dtypes.bfloat16)


def _run(inputs, trace=False, **kwargs):
    nc = build()
    x = np.asarray(inputs["x"], dtype=np.float32)
    xT = [np.ascontiguousarray(x[b].T.reshape(NEC, P, S)) for b in range(B)]
    wc, bc = _fold_weights(
        inputs["qkv_w"], inputs["qkv_b"], inputs["out_w"], inputs["out_b"]
    )
    wpk = [_pack_w(wc, h) for h in range(2)]
    bpk = [
        np.ascontiguousarray(bc[h * EH : (h + 1) * EH].reshape(1, EH)).astype(
            ml_dtypes.bfloat16
        )
        for h in range(2)
    ]
    in_maps = [
        {"xt": xT[c // 2], "w": wpk[c % 2], "b": bpk[c % 2]} for c in range(N_CORES)
    ]
    res = run_bass_kernel_spmd(
        nc, in_maps, core_ids=list(range(N_CORES)), trace=trace, **kwargs
    )
    out = np.empty((B, S, E), dtype=np.float32)
    for b in range(B):
        for h in range(2):
            o = res.results[2 * b + h]["o"]
            o = o.reshape(P, S // P, EH).transpose(1, 0, 2).reshape(S, EH)
            out[b, :, h * EH : (h + 1) * EH] = o
    return out, res


def kernel(**inputs) -> np.ndarray:
    out, _ = _run(inputs, trace=False)
    return out


# revision 16
# speedup vs baseline: 1.0959x; 1.0838x over previous
"""Trainium2 Bass kernel for nn_Attention_65128884077225.

Math: the reference module broadcasts scores [B,H,S,1] along the softmax
axis, so every softmax row is constant -> attention weights are exactly
uniform (1/S). Hence z = mean_s(v) broadcast over s, and the whole module
collapses to, per batch b:

    c[b] = (mean_s x[b,s,:]) @ Wv @ Wout + (bv @ Wout + bout)
    out[b,s,:] = c[b]                      (constant across s)

where Wv = qkv_w[:, 2E:3E], bv = qkv_b[2E:3E].

Sharding: 8 cores = 4 batches x 2 column-halves. Core c handles batch
b=c//2 and output columns [h*256, (h+1)*256), h=c%2. Each core reads the
full x[b] (needed for the mean), but only its half of the folded weight
matrix, and writes out[b][:, cols] (2 MiB).

Device kernel per core:
  - 16 DMA loads of x row-tiles [128,512], alternating across the two
    HWDGE rings (sync + scalar),
  - serial DVE add-chain accumulates the 16 tiles -> acc [128,512],
  - 4 matmuls vs a ones-vector give column sums xsum^T [128,4],
  - 4-step accumulated matmul xsum @ Wc_half -> row [1,256], + bias,
  - rank-1 matmul broadcasts the row -> [128,256] tile,
  - 16 stores of that tile cover out[b][:, cols] (written as a contiguous
    [2048,256] per-core output, reassembled on host).

Host only: fold Wc = (Wv @ Wout)/S and bc = bv @ Wout + bout (tiny host
GEMM), shard inputs, and concatenate the per-core outputs.
"""

import sys

import numpy as np

if "/opt/trn_rl_repo" not in sys.path and not any(
    p.endswith("trn_rl_repo") for p in sys.path
):
    sys.path.insert(0, "/opt/trn_rl_repo")

import concourse.bacc as bacc
import concourse.mybir as mybir
import concourse.tile as tile
from concourse.bass_utils import run_bass_kernel_spmd

B, S, E = 4, 2048, 512
N_CORES = 8
P = 128
N_XT = S // P          # 16 x-tiles of [128, 512]
EH = E // 2            # 256 output columns per core
N_OT = S // P          # 16 output tiles of [128, 256]
FP32 = mybir.dt.float32

_CACHE = {}


def build():
    """Build + compile the per-core Bass program (same for every core)."""
    if "nc" in _CACHE:
        return _CACHE["nc"]
    nc = bacc.Bacc(None, target_bir_lowering=False, enable_partition_id=False)
    x_d = nc.dram_tensor("x", [S, E], FP32, kind="ExternalInput")
    wc_d = nc.dram_tensor("wc", [E, EH], FP32, kind="ExternalInput")
    bc_d = nc.dram_tensor("bc", [EH], FP32, kind="ExternalInput")
    o_d = nc.dram_tensor("o", [S, EH], FP32, kind="ExternalOutput")

    def ring(i):
        return nc.sync if i % 2 == 0 else nc.scalar

    with tile.TileContext(nc) as tc:
        with (
            tc.tile_pool(name="xp", bufs=N_XT) as xp,
            tc.tile_pool(name="wp", bufs=4) as wp,
            tc.tile_pool(name="sp", bufs=1) as sp,
            tc.tile_pool(name="ps", bufs=1, space="PSUM") as ps,
        ):
            ones_col = sp.tile([P, 1], FP32, tag="ones_col")
            nc.vector.memset(ones_col[:], 1.0)
            ones_row = sp.tile([1, P], FP32, tag="ones_row")
            nc.vector.memset(ones_row[:], 1.0)

            # PE warm-up (HAM): dummy rank-reductions chained to late tiles
            # keep the PE clocked at 2.4 GHz going into the tail matmuls.
            p_warm = ps.tile([1, E], FP32, tag="warm")

            xts = []
            for t in range(N_XT):
                xt = xp.tile([P, E], FP32, tag="x")
                ring(t).dma_start(xt[:], x_d[t * P : (t + 1) * P, :])
                xts.append(xt)
                if 8 <= t:
                    nc.tensor.matmul(
                        p_warm[0:1, 0:EH],
                        ones_col[:],
                        xt[:, :EH],
                        start=True,
                        stop=True,
                    )

            wts = []
            for k in range(4):
                wt = wp.tile([P, EH], FP32, tag="w")
                ring(k).dma_start(wt[:], wc_d[k * P : (k + 1) * P, :])
                wts.append(wt)

            # bias row load: emitted after the x tiles so the tiny transfer
            # doesn't head-of-line-block the x stream (needed only by the
            # crow bias-add, ~26us in)
            bcr = sp.tile([1, EH], FP32, tag="bcr")
            nc.sync.dma_start(bcr[:], bc_d[None, :])
            p_bc = ps.tile([P, EH], FP32, tag="bc")

            # serial accumulate t0..t14; the final tile's add is split lo/hi
            # into separate tiles so the lo reduction+copy overlaps the hi add
            acc = sp.tile([P, E], FP32, tag="acc")
            nc.vector.tensor_add(acc[:], xts[0][:], xts[1][:])
            for t in range(2, N_XT - 1):
                nc.vector.tensor_add(acc[:], acc[:], xts[t][:])
            acc_lo = sp.tile([P, EH], FP32, tag="acc_lo")
            acc_hi = sp.tile([P, EH], FP32, tag="acc_hi")
            nc.vector.tensor_add(acc_lo[:], acc[:, :EH], xts[15][:, :EH])
            nc.vector.tensor_add(acc_hi[:], acc[:, EH:], xts[15][:, EH:])

            # column sums: xsum^T; lo wave then hi wave
            p_red = ps.tile([P, 4], FP32, tag="red")
            accs = [acc_lo, acc_lo, acc_hi, acc_hi]
            for c in range(4):
                nc.tensor.matmul(
                    p_red[:, c : c + 1],
                    accs[c][:, (c % 2) * P : (c % 2 + 1) * P],
                    ones_col[:],
                    start=True,
                    stop=True,
                )
            # lo/hi PSUM->SBUF copies on the idle ACT engine (the DVE's
            # sequencer is backed up with waits after the add chain)
            xsumT_lo = sp.tile([P, 2], FP32, tag="xsumT_lo")
            nc.scalar.copy(xsumT_lo[:], p_red[:, 0:2])
            xsumT_hi = sp.tile([P, 2], FP32, tag="xsumT_hi")
            nc.scalar.copy(xsumT_hi[:], p_red[:, 2:4])

            # c_row [1, 256] = xsum @ Wc_half  (accumulate over 4 k-chunks)
            p_crow = ps.tile([1, EH], FP32, tag="crow")
            xTs = [xsumT_lo, xsumT_lo, xsumT_hi, xsumT_hi]
            for k in range(4):
                nc.tensor.matmul(
                    p_crow[:],
                    xTs[k][:, k % 2 : k % 2 + 1],
                    wts[k][:],
                    start=(k == 0),
                    stop=(k == 3),
                )
            crow = sp.tile([1, EH], FP32, tag="crowsb")
            nc.vector.tensor_add(crow[:], p_crow[:], bcr[:])

            # broadcast row across partitions via rank-1 matmul
            nc.tensor.matmul(p_bc[:], ones_row[:], crow[:], start=True, stop=True)
            bcast = sp.tile([P, EH], FP32, tag="bcast")
            nc.vector.tensor_copy(bcast[:], p_bc[:])

            # 4 stores, each covering 512 output rows via stride-0 source
            o_t = o_d.rearrange("(t p) e -> p t e", p=P)
            src = bcast[:, None, :].broadcast_to([P, 4, EH])
            for u in range(4):
                ring(u).dma_start(o_t[:, 4 * u : 4 * (u + 1), :], src)

    nc.compile()
    _CACHE["nc"] = nc
    return nc


def _fold_weights(qkv_w, qkv_b, out_w, out_b):
    wv = np.asarray(qkv_w)[:, 2 * E : 3 * E].astype(np.float64)
    wc = (wv @ np.asarray(out_w).astype(np.float64) / S).astype(np.float32)
    bc = (
        np.asarray(qkv_b)[2 * E : 3 * E].astype(np.float64)
        @ np.asarray(out_w).astype(np.float64)
        + np.asarray(out_b)
    ).astype(np.float32)
    return wc, bc


def _run(inputs, trace=False, **kwargs):
    nc = build()
    x = np.ascontiguousarray(np.asarray(inputs["x"], dtype=np.float32))
    wc, bc = _fold_weights(
        inputs["qkv_w"], inputs["qkv_b"], inputs["out_w"], inputs["out_b"]
    )
    in_maps = [
        {
            "x": x[c // 2],
            "wc": np.ascontiguousarray(wc[:, (c % 2) * EH : (c % 2 + 1) * EH]),
            "bc": np.ascontiguousarray(bc[(c % 2) * EH : (c % 2 + 1) * EH]),
        }
        for c in range(N_CORES)
    ]
    res = run_bass_kernel_spmd(
        nc, in_maps, core_ids=list(range(N_CORES)), trace=trace, **kwargs
    )
    out = np.empty((B, S, E), dtype=np.float32)
    for b in range(B):
        out[b, :, :EH] = res.results[2 * b]["o"]
        out[b, :, EH:] = res.results[2 * b + 1]["o"]
    return out, res


def kernel(**inputs) -> np.ndarray:
    out, _ = _run(inputs, trace=False)
    return out


# revision 21
# speedup vs baseline: 1.1061x; 1.0093x over previous
"""Trainium2 Bass kernel for nn_Attention_65128884077225.

Math: the reference module broadcasts scores [B,H,S,1] along the softmax
axis, so every softmax row is constant -> attention weights are exactly
uniform (1/S). Hence z = mean_s(v) broadcast over s, and the whole module
collapses to, per batch b:

    c[b] = (mean_s x[b,s,:]) @ Wv @ Wout + (bv @ Wout + bout)
    out[b,s,:] = c[b]                      (constant across s)

where Wv = qkv_w[:, 2E:3E], bv = qkv_b[2E:3E].

Sharding: 8 cores = 4 batches x 2 column-halves. Core c handles batch
b=c//2 and output columns [h*256, (h+1)*256), h=c%2. Each core reads the
full x[b] (needed for the mean), but only its half of the folded weight
matrix, and writes out[b][:, cols] (2 MiB).

Device kernel per core:
  - 16 DMA loads of x row-tiles [128,512], alternating across the two
    HWDGE rings (sync + scalar),
  - serial DVE add-chain accumulates the 16 tiles -> acc [128,512],
  - 4 matmuls vs a ones-vector give column sums xsum^T [128,4],
  - 4-step accumulated matmul xsum @ Wc_half -> row [1,256], + bias,
  - rank-1 matmul broadcasts the row -> [128,256] tile,
  - 16 stores of that tile cover out[b][:, cols] (written as a contiguous
    [2048,256] per-core output, reassembled on host).

Host only: fold Wc = (Wv @ Wout)/S and bc = bv @ Wout + bout (tiny host
GEMM), shard inputs, and concatenate the per-core outputs.
"""

import sys

import numpy as np

if "/opt/trn_rl_repo" not in sys.path and not any(
    p.endswith("trn_rl_repo") for p in sys.path
):
    sys.path.insert(0, "/opt/trn_rl_repo")

import ml_dtypes

import concourse.bacc as bacc
import concourse.mybir as mybir
import concourse.tile as tile
from concourse.bass_utils import run_bass_kernel_spmd

B, S, E = 4, 2048, 512
N_CORES = 8
P = 128
N_XT = S // P          # 16 x-tiles of [128, 512]
EH = E // 2            # 256 output columns per core
NEC = E // P           # 4 contraction chunks for the crow GEMV
BCAST_Q = 4            # SBUF-side replication of the out tile
FP32 = mybir.dt.float32
BF16 = mybir.dt.bfloat16

_CACHE = {}


def build():
    """Build + compile the per-core Bass program (same for every core)."""
    if "nc" in _CACHE:
        return _CACHE["nc"]
    nc = bacc.Bacc(None, target_bir_lowering=False, enable_partition_id=False)
    x_d = nc.dram_tensor("x", [S, E], FP32, kind="ExternalInput")
    w_d = nc.dram_tensor("w", [P, NEC * EH], BF16, kind="ExternalInput")
    b_d = nc.dram_tensor("b", [1, EH], BF16, kind="ExternalInput")
    o_d = nc.dram_tensor("o", [P, (S // P) * EH], FP32, kind="ExternalOutput")

    def ring(i):
        return nc.sync if i % 2 == 0 else nc.scalar

    with tile.TileContext(nc) as tc:
        with (
            tc.tile_pool(name="xp", bufs=N_XT) as xp,
            tc.tile_pool(name="wp", bufs=1) as wp,
            tc.tile_pool(name="sp", bufs=1) as sp,
            tc.tile_pool(name="ps", bufs=1, space="PSUM") as ps,
        ):
            ones_col = sp.tile([P, 1], FP32, tag="ones_col")
            nc.vector.memset(ones_col[:], 1.0)
            ones2 = sp.tile([2, P], BF16, tag="ones2")
            nc.vector.memset(ones2[:], 1.0)

            # PE warm-up (HAM): dummy rank-reductions chained to late tiles
            # keep the PE clocked at 2.4 GHz going into the tail matmuls.
            p_warm = ps.tile([1, E], FP32, tag="warm")

            xts = []
            for t in range(N_XT):
                xt = xp.tile([P, E], FP32, tag="x")
                ring(t).dma_start(xt[:], x_d[t * P : (t + 1) * P, :])
                xts.append(xt)
                if 8 <= t:
                    nc.tensor.matmul(
                        p_warm[0:1, 0:EH],
                        ones_col[:],
                        xt[:, :EH],
                        start=True,
                        stop=True,
                    )

            # weights (bf16, E-chunk-major) + bias after the x tiles so the
            # small transfers don't head-of-line-block the x stream; both
            # are needed only by the tail (~24us in)
            wcb = wp.tile([P, NEC * EH], BF16, tag="w")
            nc.scalar.dma_start(wcb[:], w_d[:, :])
            # cb row 0 <- crow (copied from PSUM later); row 1 <- bias DMA.
            # The k=2 broadcast matmul then adds the bias for free.
            cb = sp.tile([2, EH], BF16, tag="cb")
            nc.sync.dma_start(cb[1:2, :], b_d[:, :])

            # serial accumulate t0..t14; the final tile's add is split lo/hi
            # into separate tiles so the lo reduction+copy overlaps the hi add
            acc = sp.tile([P, E], FP32, tag="acc")
            nc.vector.tensor_add(acc[:], xts[0][:], xts[1][:])
            for t in range(2, N_XT - 1):
                nc.vector.tensor_add(acc[:], acc[:], xts[t][:])
            acc_lo = sp.tile([P, EH], FP32, tag="acc_lo")
            acc_hi = sp.tile([P, EH], FP32, tag="acc_hi")
            nc.vector.tensor_add(acc_lo[:], acc[:, :EH], xts[15][:, :EH])
            nc.vector.tensor_add(acc_hi[:], acc[:, EH:], xts[15][:, EH:])

            # column sums: xsum^T; lo wave then hi wave
            p_red = ps.tile([P, 4], FP32, tag="red")
            accs = [acc_lo, acc_lo, acc_hi, acc_hi]
            for c in range(4):
                nc.tensor.matmul(
                    p_red[:, c : c + 1],
                    accs[c][:, (c % 2) * P : (c % 2 + 1) * P],
                    ones_col[:],
                    start=True,
                    stop=True,
                )
            # lo/hi PSUM->SBUF copies (cast to bf16) on the idle ACT engine
            # (the DVE's sequencer is backed up with waits after the adds)
            xsumT_lo = sp.tile([P, 2], BF16, tag="xsumT_lo")
            nc.scalar.copy(xsumT_lo[:], p_red[:, 0:2])
            xsumT_hi = sp.tile([P, 2], BF16, tag="xsumT_hi")
            nc.scalar.copy(xsumT_hi[:], p_red[:, 2:4])

            # c_row [1, 256] = xsum @ Wc_half (bf16, accumulate 4 k-chunks)
            p_crow = ps.tile([1, EH], FP32, tag="crow")
            xTs = [xsumT_lo, xsumT_lo, xsumT_hi, xsumT_hi]
            for k in range(4):
                nc.tensor.matmul(
                    p_crow[:],
                    xTs[k][:, k % 2 : k % 2 + 1],
                    wcb[:, k * EH : (k + 1) * EH],
                    start=(k == 0),
                    stop=(k == 3),
                )
            nc.scalar.copy(cb[0:1, :], p_crow[:])

            # broadcast crow+bias across partitions via k=2 matmuls into two
            # PSUM banks so DVE and ACT replicate in parallel afterwards
            p_bc0 = ps.tile([P, EH], FP32, tag="bc0")
            p_bc1 = ps.tile([P, EH], FP32, tag="bc1")
            nc.tensor.matmul(p_bc0[:], ones2[:], cb[:], start=True, stop=True)
            nc.tensor.matmul(p_bc1[:], ones2[:], cb[:], start=True, stop=True)
            bcast = sp.tile([P, BCAST_Q, EH], FP32, tag="bcast")
            nc.vector.tensor_copy(
                bcast[:, 0:2, :], p_bc0[:, None, :].broadcast_to([P, 2, EH])
            )
            nc.scalar.copy(bcast[:, 2, :], p_bc1[:, :])
            nc.scalar.copy(bcast[:, 3, :], p_bc1[:, :])

            # ONE store: p-major dst (16 KiB/partition contiguous), source
            # replicated x4 -> 4 KiB descriptors
            o_t = o_d.rearrange("p (g q e) -> p g (q e)", q=BCAST_Q, e=EH)
            src = bcast[:, None, :, :].broadcast_to(
                [P, (S // P) // BCAST_Q, BCAST_Q, EH]
            ).rearrange("p g q e -> p g (q e)")
            nc.sync.dma_start(o_t[:, :, :], src)

    nc.compile()
    _CACHE["nc"] = nc
    return nc


def _fold_weights(qkv_w, qkv_b, out_w, out_b):
    wv = np.asarray(qkv_w)[:, 2 * E : 3 * E].astype(np.float64)
    wc = (wv @ np.asarray(out_w).astype(np.float64) / S).astype(np.float32)
    bc = (
        np.asarray(qkv_b)[2 * E : 3 * E].astype(np.float64)
        @ np.asarray(out_w).astype(np.float64)
        + np.asarray(out_b)
    ).astype(np.float32)
    return wc, bc


def _pack_w(wc, h):
    """[128, 4*256] bf16: E-chunk-major packing of this half's Wc columns."""
    cols = slice(h * EH, (h + 1) * EH)
    return np.ascontiguousarray(
        wc[:, cols].reshape(NEC, P, EH).transpose(1, 0, 2).reshape(P, NEC * EH)
    ).astype(ml_dtypes.bfloat16)


def _run(inputs, trace=False, **kwargs):
    nc = build()
    x = np.ascontiguousarray(np.asarray(inputs["x"], dtype=np.float32))
    wc, bc = _fold_weights(
        inputs["qkv_w"], inputs["qkv_b"], inputs["out_w"], inputs["out_b"]
    )
    wpk = [_pack_w(wc, h) for h in range(2)]
    bpk = [
        np.ascontiguousarray(bc[h * EH : (h + 1) * EH].reshape(1, EH)).astype(
            ml_dtypes.bfloat16
        )
        for h in range(2)
    ]
    in_maps = [
        {"x": x[c // 2], "w": wpk[c % 2], "b": bpk[c % 2]} for c in range(N_CORES)
    ]
    res = run_bass_kernel_spmd(
        nc, in_maps, core_ids=list(range(N_CORES)), trace=trace, **kwargs
    )
    out = np.empty((B, S, E), dtype=np.float32)
    for b in range(B):
        for h in range(2):
            o = res.results[2 * b + h]["o"]
            o = o.reshape(P, S // P, EH).transpose(1, 0, 2).reshape(S, EH)
            out[b, :, h * EH : (h + 1) * EH] = o
    return out, res


def kernel(**inputs) -> np.ndarray:
    out, _ = _run(inputs, trace=False)
    return out
